# revision 4
# baseline (speedup 1.0000x reference)
"""MetaQDA Trainium2 kernel.

Strategy: Woodbury + matrix-determinant-lemma reformulation done on host
(fp64, tiny: one shared 768x768 inverse + 64 18x18 systems), reducing the
device work to dense matmuls per query shard:

  per query column q (768-vec):
    SQ   = M1 @ q                  (M1 [1920,768] = class rank-update rows + chol(P)^T)
    F[c] = Sel^T SQ^2 + Sel2^T q^2 + Coef^T q     (PSUM accumulation, [64] per q)
    out[c] = biasc[c] + lnmul[c] * ln(F[c]*invc[c] + blc[c])

Queries are sharded 8-way across NeuronCores (256 each); all class data is
replicated (it is only ~6.5MB of fp32 matrices). No collectives.
"""
import sys
import numpy as np
from math import lgamma

sys.path.insert(0, "/opt/trn_rl_repo")

REG = 0.5
D = 768
C = 64
NQ = 2048
NCORES = 8
QC = NQ // NCORES          # 256 queries per core
KPAD = 1152                # sum(1+Nc) = C+N = 1088, padded to 9*128
M1ROWS = KPAD + D          # 1920
KT_M1 = D // 128           # 6 contraction tiles for M1 @ q
MT_M1 = M1ROWS // 128      # 15 output tiles of SQ
SELTILES = M1ROWS // 128   # 15
F32 = np.float32


# ---------------------------------------------------------------- host math
def _precompute(support_X, query_X, m, kappa, nu, triu_diag, triu_lower,
                labels, way):
    d = D
    f64 = np.float64
    Xs = np.asarray(support_X, f64)
    mv = np.asarray(m, f64).reshape(-1)
    kap = abs(float(kappa)) + 1e-6
    nu_ = max(float(nu), d - 1 + 1e-6)
    way = int(way)

    Ld = np.abs(np.asarray(triu_diag, f64))
    L = np.diag(Ld) + np.tril(np.asarray(triu_lower, f64), -1)
    logdet_prior = 2.0 * np.sum(np.log(Ld))

    # P = inv(L L^T) via triangular solve in fp64
    Linv = _solve_tri_lower(L, np.eye(d))
    P = Linv.T @ Linv

    labels = np.asarray(labels).astype(np.int64)
    counts = np.bincount(labels, minlength=way).astype(f64)
    sums = np.zeros((way, d), f64)
    np.add.at(sums, labels, Xs)
    starts = np.searchsorted(labels, np.arange(way), side="left")
    ends = np.searchsorted(labels, np.arange(way), side="right")

    v = kap * mv[None, :] + sums
    mu = v / (kap + counts)[:, None]
    scale = (kap + counts + 1.0) / ((nu_ + counts - d + 1.0) * (kap + counts))
    alpha = (1.0 - REG) / scale
    common = nu_ + counts + 1.0 - d

    Pmu = mu @ P
    KXa = np.zeros((KPAD, d), f64)
    SelT = np.zeros((M1ROWS, way), f64)
    Sel2 = np.full((D, way), REG, f64)
    CoefT = np.zeros((d, way), f64)
    delta = np.zeros(way, f64)
    logdet = np.zeros(way, f64)

    row0 = 0
    for c in range(way):
        Nc = int(counts[c])
        Xc = Xs[starts[c]:ends[c]]
        nb = Nc + 2
        W = np.concatenate([mv[:, None], Xc.T, v[c][:, None]], axis=1)
        PW = P @ W
        Jdiag = np.concatenate([[kap], np.ones(Nc), [-1.0 / (kap + Nc)]])
        Mc = np.diag(1.0 / Jdiag) + W.T @ PW
        E = np.linalg.inv(Mc)
        _, ldM = np.linalg.slogdet(Mc)
        logdet[c] = (d * np.log(scale[c]) + logdet_prior + ldM
                     + np.sum(np.log(np.abs(Jdiag))))
        h = PW.T @ mu[c]
        Eh = E @ h
        T = np.zeros((nb, 1 + Nc))
        T[0, 0] = 1.0
        T[1:1 + Nc, 1:1 + Nc] = np.eye(Nc)
        T[nb - 1, 0] = kap
        T[nb - 1, 1:1 + Nc] = 1.0
        Fq = T.T @ E @ T
        g = T.T @ Eh
        kc = h @ Eh
        lam, V = np.linalg.eigh(Fq)
        R = (np.sqrt(np.abs(lam))[:, None]) * V.T
        Xhat = np.concatenate([mv[None, :], Xc], axis=0)
        KXa[row0:row0 + 1 + Nc] = (R @ Xhat) @ P
        SelT[row0:row0 + 1 + Nc, c] = -alpha[c] * np.sign(lam)
        row0 += 1 + Nc
        ghat = Xhat.T @ g
        CoefT[:, c] = (-2.0 * REG * mu[c] - 2.0 * alpha[c] * Pmu[c]
                       + 2.0 * alpha[c] * (P @ ghat))
        delta[c] = alpha[c] * (mu[c] @ Pmu[c] - kc) + REG * (mu[c] @ mu[c])

    Lp = np.linalg.cholesky(P)
    SelT[KPAD:KPAD + D, :] = alpha[None, :]
    M1 = np.concatenate([KXa, Lp.T], axis=0)

    biasc = np.array([lgamma(0.5 * (common[c] + d)) - lgamma(0.5 * common[c])
                      - 0.5 * d * np.log(common[c]) - 0.5 * logdet[c]
                      for c in range(way)])
    invc = 1.0 / common
    lnmul = -0.5 * (common + d)
    blc = 1.0 + delta * invc

    return dict(
        M1T=np.ascontiguousarray(M1.T, dtype=F32),
        SelT=np.ascontiguousarray(SelT, dtype=F32),
        Sel2=np.ascontiguousarray(Sel2, dtype=F32),
        CoefT=np.ascontiguousarray(CoefT, dtype=F32),
        PC=np.ascontiguousarray(
            np.stack([invc, lnmul, biasc, blc], axis=1), dtype=F32),
    )


def _solve_tri_lower(L, B):
    try:
        import scipy.linalg as sla
        return sla.solve_triangular(L, B, lower=True)
    except ImportError:
        return np.linalg.solve(L, B)


# ---------------------------------------------------------------- device
_CACHE = {}


def _build_program():
    import concourse.bass as bass
    import concourse.bacc as bacc
    import concourse.tile as tile
    from concourse import mybir

    nc = bacc.Bacc("TRN2", target_bir_lowering=False, debug=False,
                   num_devices=NCORES)
    dt = mybir.dt.float32
    qT = nc.dram_tensor("qT", [D, QC], dt, kind="ExternalInput").ap()
    M1T = nc.dram_tensor("M1T", [D, M1ROWS], dt, kind="ExternalInput").ap()
    SelT = nc.dram_tensor("SelT", [M1ROWS, C], dt, kind="ExternalInput").ap()
    Sel2 = nc.dram_tensor("Sel2", [D, C], dt, kind="ExternalInput").ap()
    CoefT = nc.dram_tensor("CoefT", [D, C], dt, kind="ExternalInput").ap()
    PC = nc.dram_tensor("PC", [C, 4], dt, kind="ExternalInput").ap()
    out = nc.dram_tensor("out", [C, QC], dt, kind="ExternalOutput").ap()

    with tile.TileContext(nc) as tc:
        with (
            tc.tile_pool(name="qpool", bufs=1) as qpool,
            tc.tile_pool(name="wpool", bufs=1) as wpool,
            tc.tile_pool(name="sqpool", bufs=1) as sqpool,
            tc.tile_pool(name="psum", bufs=4, space="PSUM") as psum,
            tc.tile_pool(name="fpsum", bufs=1, space="PSUM") as fpsum,
            tc.tile_pool(name="opool", bufs=1) as opool,
        ):
            # load q^T tiles [128, QC] x 6
            qts = []
            for k in range(KT_M1):
                t = qpool.tile([128, QC], dt, tag=f"qt{k}")
                nc.sync.dma_start(t[:], qT[k * 128:(k + 1) * 128, :])
                qts.append(t)
            # weights
            m1ts = []
            for k in range(KT_M1):
                t = wpool.tile([128, M1ROWS], dt, tag=f"m1t{k}")
                nc.sync.dma_start(t[:], M1T[k * 128:(k + 1) * 128, :])
                m1ts.append(t)
            selts = []
            for j in range(SELTILES):
                t = wpool.tile([128, C], dt, tag=f"sel{j}")
                nc.sync.dma_start(t[:], SelT[j * 128:(j + 1) * 128, :])
                selts.append(t)
            sel2ts = []
            coefts = []
            for k in range(KT_M1):
                t = wpool.tile([128, C], dt, tag=f"sel2_{k}")
                nc.sync.dma_start(t[:], Sel2[k * 128:(k + 1) * 128, :])
                sel2ts.append(t)
                t2 = wpool.tile([128, C], dt, tag=f"coef{k}")
                nc.sync.dma_start(t2[:], CoefT[k * 128:(k + 1) * 128, :])
                coefts.append(t2)
            pct = wpool.tile([C, 4], dt, tag="pc")
            nc.sync.dma_start(pct[:], PC[:])

            # F psum [64, QC] accumulated across all matmuls
            fac = fpsum.tile([C, QC], dt, tag="facc")
            first = [True]

            def acc(lhsT, rhs, stop=False):
                nc.tensor.matmul(fac[:], lhsT, rhs, start=first[0], stop=stop)
                first[0] = False

            # SQ tiles: for each of 15 output tiles, 6-step contraction
            sqs = []
            for mtile in range(MT_M1):
                pt = psum.tile([128, QC], dt, tag="sqp")
                for k in range(KT_M1):
                    nc.tensor.matmul(
                        pt[:],
                        m1ts[k][:, mtile * 128:(mtile + 1) * 128],
                        qts[k][:],
                        start=(k == 0), stop=(k == KT_M1 - 1),
                    )
                sq = sqpool.tile([128, QC], dt, tag=f"sqsq{mtile}")
                nc.scalar.square(sq[:], pt[:])
                sqs.append(sq)

            # q^2 rows (DVE)
            q2s = []
            for k in range(KT_M1):
                q2 = sqpool.tile([128, QC], dt, tag=f"q2_{k}")
                nc.vector.tensor_mul(q2[:], qts[k][:], qts[k][:])
                q2s.append(q2)

            # accumulate everything into F
            for mtile in range(MT_M1):
                acc(selts[mtile][:], sqs[mtile][:])
            for k in range(KT_M1):
                acc(sel2ts[k][:], q2s[k][:])
            for k in range(KT_M1):
                acc(coefts[k][:], qts[k][:], stop=(k == KT_M1 - 1))

            # epilogue: out = biasc + lnmul * ln(F*invc + blc)
            lnt = opool.tile([C, QC], dt, tag="lnt")
            nc.scalar.activation(lnt[:], fac[:],
                                 mybir.ActivationFunctionType.Ln,
                                 bias=pct[:, 3:4], scale=pct[:, 0:1])
            ot = opool.tile([C, QC], dt, tag="ot")
            nc.vector.tensor_scalar(ot[:], lnt[:], pct[:, 1:2], pct[:, 2:3],
                                    op0=mybir.AluOpType.mult,
                                    op1=mybir.AluOpType.add)
            nc.sync.dma_start(out[:], ot[:])

    nc.compile()
    return nc


def _run(inputs, **run_kwargs):
    from concourse.bass_utils import run_bass_kernel_spmd

    pre = _precompute(**inputs)
    if "nc" not in _CACHE:
        _CACHE["nc"] = _build_program()
    nc = _CACHE["nc"]

    qT_full = np.ascontiguousarray(np.asarray(inputs["query_X"], F32).T)
    in_maps = []
    for i in range(NCORES):
        im = dict(pre)
        im["qT"] = np.ascontiguousarray(qT_full[:, i * QC:(i + 1) * QC])
        in_maps.append(im)

    res = run_bass_kernel_spmd(nc, in_maps, core_ids=list(range(NCORES)),
                               **run_kwargs)
    outs = [res.results[i]["out"] for i in range(NCORES)]     # [64, 256] each
    full = np.concatenate(outs, axis=1)                        # [64, 2048]
    return np.ascontiguousarray(full.T.astype(F32)), res       # [2048, 64]


def kernel(support_X, query_X, m, kappa, nu, triu_diag, triu_lower,
           labels, way):
    out, _ = _run(dict(
        support_X=support_X, query_X=query_X, m=m, kappa=kappa, nu=nu,
        triu_diag=triu_diag, triu_lower=triu_lower, labels=labels, way=way))
    return out


# revision 7
# speedup vs baseline: 1.4891x; 1.4891x over previous
"""MetaQDA Trainium2 kernel.

Strategy: Woodbury + matrix-determinant-lemma reformulation done on host
(fp64, tiny: one shared 768x768 inverse + 64 18x18 systems), reducing the
device work to dense matmuls per query shard:

  per query column q (768-vec):
    SQ   = M1 @ q                  (M1 [1920,768] = class rank-update rows + chol(P)^T)
    F[c] = Sel^T SQ^2 + Sel2^T q^2 + Coef^T q     (PSUM accumulation, [64] per q)
    out[c] = biasc[c] + lnmul[c] * ln(F[c]*invc[c] + blc[c])

Queries are sharded 8-way across NeuronCores (256 each); all class data is
replicated (it is only ~6.5MB of fp32 matrices). No collectives.
"""
import sys
import numpy as np
from math import lgamma

sys.path.insert(0, "/opt/trn_rl_repo")

REG = 0.5
D = 768
C = 64
NQ = 2048
NCORES = 8
QC = NQ // NCORES          # 256 queries per core
KPAD = 1152                # sum(1+Nc) = C+N = 1088, padded to 9*128
M1ROWS = KPAD + D          # 1920
KT_M1 = D // 128           # 6 contraction tiles for M1 @ q
MT_M1 = M1ROWS // 128      # 15 output tiles of SQ
SELTILES = M1ROWS // 128   # 15
F32 = np.float32


# ---------------------------------------------------------------- host math
def _precompute(support_X, query_X, m, kappa, nu, triu_diag, triu_lower,
                labels, way):
    d = D
    f64 = np.float64
    Xs = np.asarray(support_X, f64)
    mv = np.asarray(m, f64).reshape(-1)
    kap = abs(float(kappa)) + 1e-6
    nu_ = max(float(nu), d - 1 + 1e-6)
    way = int(way)

    Ld = np.abs(np.asarray(triu_diag, f64))
    L = np.diag(Ld) + np.tril(np.asarray(triu_lower, f64), -1)
    logdet_prior = 2.0 * np.sum(np.log(Ld))

    # P = inv(L L^T) via triangular solve in fp64
    Linv = _solve_tri_lower(L, np.eye(d))
    P = Linv.T @ Linv

    labels = np.asarray(labels).astype(np.int64)
    counts = np.bincount(labels, minlength=way).astype(f64)
    sums = np.zeros((way, d), f64)
    np.add.at(sums, labels, Xs)
    starts = np.searchsorted(labels, np.arange(way), side="left")
    ends = np.searchsorted(labels, np.arange(way), side="right")

    v = kap * mv[None, :] + sums
    mu = v / (kap + counts)[:, None]
    scale = (kap + counts + 1.0) / ((nu_ + counts - d + 1.0) * (kap + counts))
    alpha = (1.0 - REG) / scale
    common = nu_ + counts + 1.0 - d

    Pmu = mu @ P
    KXa = np.zeros((KPAD, d), f64)
    SelT = np.zeros((M1ROWS, way), f64)
    Sel2 = np.full((D, way), REG, f64)
    CoefT = np.zeros((d, way), f64)
    delta = np.zeros(way, f64)
    logdet = np.zeros(way, f64)

    row0 = 0
    for c in range(way):
        Nc = int(counts[c])
        Xc = Xs[starts[c]:ends[c]]
        nb = Nc + 2
        W = np.concatenate([mv[:, None], Xc.T, v[c][:, None]], axis=1)
        PW = P @ W
        Jdiag = np.concatenate([[kap], np.ones(Nc), [-1.0 / (kap + Nc)]])
        Mc = np.diag(1.0 / Jdiag) + W.T @ PW
        E = np.linalg.inv(Mc)
        _, ldM = np.linalg.slogdet(Mc)
        logdet[c] = (d * np.log(scale[c]) + logdet_prior + ldM
                     + np.sum(np.log(np.abs(Jdiag))))
        h = PW.T @ mu[c]
        Eh = E @ h
        T = np.zeros((nb, 1 + Nc))
        T[0, 0] = 1.0
        T[1:1 + Nc, 1:1 + Nc] = np.eye(Nc)
        T[nb - 1, 0] = kap
        T[nb - 1, 1:1 + Nc] = 1.0
        Fq = T.T @ E @ T
        g = T.T @ Eh
        kc = h @ Eh
        lam, V = np.linalg.eigh(Fq)
        R = (np.sqrt(np.abs(lam))[:, None]) * V.T
        Xhat = np.concatenate([mv[None, :], Xc], axis=0)
        KXa[row0:row0 + 1 + Nc] = (R @ Xhat) @ P
        SelT[row0:row0 + 1 + Nc, c] = -alpha[c] * np.sign(lam)
        row0 += 1 + Nc
        ghat = Xhat.T @ g
        CoefT[:, c] = (-2.0 * REG * mu[c] - 2.0 * alpha[c] * Pmu[c]
                       + 2.0 * alpha[c] * (P @ ghat))
        delta[c] = alpha[c] * (mu[c] @ Pmu[c] - kc) + REG * (mu[c] @ mu[c])

    Lp = np.linalg.cholesky(P)
    SelT[KPAD:KPAD + D, :] = alpha[None, :]
    M1 = np.concatenate([KXa, Lp.T], axis=0)

    biasc = np.array([lgamma(0.5 * (common[c] + d)) - lgamma(0.5 * common[c])
                      - 0.5 * d * np.log(common[c]) - 0.5 * logdet[c]
                      for c in range(way)])
    invc = 1.0 / common
    lnmul = -0.5 * (common + d)
    blc = 1.0 + delta * invc

    return dict(
        M1T=np.ascontiguousarray(M1.T, dtype=F32),
        SelT=np.ascontiguousarray(SelT, dtype=F32),
        Sel2=np.ascontiguousarray(Sel2, dtype=F32),
        CoefT=np.ascontiguousarray(CoefT, dtype=F32),
        PC=np.ascontiguousarray(
            np.stack([invc, lnmul, biasc, blc], axis=1), dtype=F32),
    )


def _solve_tri_lower(L, B):
    try:
        import scipy.linalg as sla
        return sla.solve_triangular(L, B, lower=True)
    except ImportError:
        return np.linalg.solve(L, B)


# ---------------------------------------------------------------- device
_CACHE = {}


def _build_program():
    import concourse.bass as bass
    import concourse.bacc as bacc
    import concourse.tile as tile
    from concourse import mybir

    nc = bacc.Bacc("TRN2", target_bir_lowering=False, debug=False,
                   num_devices=NCORES)
    dt = mybir.dt.float32
    dtr = mybir.dt.float32r
    qT = nc.dram_tensor("qT", [D, QC], dt, kind="ExternalInput").ap()
    M1T = nc.dram_tensor("M1T", [D, M1ROWS], dt, kind="ExternalInput").ap()
    SelT = nc.dram_tensor("SelT", [M1ROWS, C], dt, kind="ExternalInput").ap()
    Sel2 = nc.dram_tensor("Sel2", [D, C], dt, kind="ExternalInput").ap()
    CoefT = nc.dram_tensor("CoefT", [D, C], dt, kind="ExternalInput").ap()
    PC = nc.dram_tensor("PC", [C, 4], dt, kind="ExternalInput").ap()
    out = nc.dram_tensor("out", [C, QC], dt, kind="ExternalOutput").ap()

    with tile.TileContext(nc) as tc:
        with (
            tc.tile_pool(name="qpool", bufs=1) as qpool,
            tc.tile_pool(name="wpool", bufs=1) as wpool,
            tc.tile_pool(name="sqpool", bufs=1) as sqpool,
            tc.tile_pool(name="psum", bufs=4, space="PSUM") as psum,
            tc.tile_pool(name="fpsum", bufs=1, space="PSUM") as fpsum,
            tc.tile_pool(name="opool", bufs=1) as opool,
        ):
            # load q^T tiles [128, QC] x 6
            qts = []
            for k in range(KT_M1):
                t = qpool.tile([128, QC], dtr, tag=f"qt{k}")
                nc.sync.dma_start(t[:], qT[k * 128:(k + 1) * 128, :].bitcast(dtr))
                qts.append(t)
            # weights
            m1ts = []
            for k in range(KT_M1):
                t = wpool.tile([128, M1ROWS], dtr, tag=f"m1t{k}")
                nc.sync.dma_start(t[:], M1T[k * 128:(k + 1) * 128, :].bitcast(dtr))
                m1ts.append(t)
            selts = []
            for j in range(SELTILES):
                t = wpool.tile([128, C], dtr, tag=f"sel{j}")
                nc.sync.dma_start(t[:], SelT[j * 128:(j + 1) * 128, :].bitcast(dtr))
                selts.append(t)
            sel2ts = []
            coefts = []
            for k in range(KT_M1):
                t = wpool.tile([128, C], dtr, tag=f"sel2_{k}")
                nc.sync.dma_start(t[:], Sel2[k * 128:(k + 1) * 128, :].bitcast(dtr))
                sel2ts.append(t)
                t2 = wpool.tile([128, C], dtr, tag=f"coef{k}")
                nc.sync.dma_start(t2[:], CoefT[k * 128:(k + 1) * 128, :].bitcast(dtr))
                coefts.append(t2)
            pct = wpool.tile([C, 4], dt, tag="pc")
            nc.sync.dma_start(pct[:], PC[:])

            # F psum [64, QC] accumulated across all matmuls
            fac = fpsum.tile([C, QC], dt, tag="facc")
            first = [True]

            def acc(lhsT, rhs, stop=False):
                nc.tensor.matmul(fac[:], lhsT, rhs, start=first[0], stop=stop)
                first[0] = False

            # SQ tiles: for each of 15 output tiles, 6-step contraction
            sqs = []
            for mtile in range(MT_M1):
                pt = psum.tile([128, QC], dt, tag="sqp")
                for k in range(KT_M1):
                    nc.tensor.matmul(
                        pt[:],
                        m1ts[k][:, mtile * 128:(mtile + 1) * 128],
                        qts[k][:],
                        start=(k == 0), stop=(k == KT_M1 - 1),
                    )
                sq = sqpool.tile([128, QC], dtr, tag=f"sqsq{mtile}")
                nc.scalar.square(sq[:], pt[:])
                sqs.append(sq)

            # q^2 rows (DVE)
            q2s = []
            for k in range(KT_M1):
                q2 = sqpool.tile([128, QC], dtr, tag=f"q2_{k}")
                nc.vector.tensor_mul(q2[:], qts[k][:], qts[k][:])
                q2s.append(q2)

            # accumulate everything into F
            for mtile in range(MT_M1):
                acc(selts[mtile][:], sqs[mtile][:])
            for k in range(KT_M1):
                acc(sel2ts[k][:], q2s[k][:])
            for k in range(KT_M1):
                acc(coefts[k][:], qts[k][:], stop=(k == KT_M1 - 1))

            # epilogue: out = biasc + lnmul * ln(F*invc + blc)
            lnt = opool.tile([C, QC], dt, tag="lnt")
            nc.scalar.activation(lnt[:], fac[:],
                                 mybir.ActivationFunctionType.Ln,
                                 bias=pct[:, 3:4], scale=pct[:, 0:1])
            ot = opool.tile([C, QC], dt, tag="ot")
            nc.vector.tensor_scalar(ot[:], lnt[:], pct[:, 1:2], pct[:, 2:3],
                                    op0=mybir.AluOpType.mult,
                                    op1=mybir.AluOpType.add)
            nc.sync.dma_start(out[:], ot[:])

    nc.compile()
    return nc


def _run(inputs, **run_kwargs):
    from concourse.bass_utils import run_bass_kernel_spmd

    pre = _precompute(**inputs)
    if "nc" not in _CACHE:
        _CACHE["nc"] = _build_program()
    nc = _CACHE["nc"]

    qT_full = np.ascontiguousarray(np.asarray(inputs["query_X"], F32).T)
    in_maps = []
    for i in range(NCORES):
        im = dict(pre)
        im["qT"] = np.ascontiguousarray(qT_full[:, i * QC:(i + 1) * QC])
        in_maps.append(im)

    res = run_bass_kernel_spmd(nc, in_maps, core_ids=list(range(NCORES)),
                               **run_kwargs)
    outs = [res.results[i]["out"] for i in range(NCORES)]     # [64, 256] each
    full = np.concatenate(outs, axis=1)                        # [64, 2048]
    return np.ascontiguousarray(full.T.astype(F32)), res       # [2048, 64]


def kernel(support_X, query_X, m, kappa, nu, triu_diag, triu_lower,
           labels, way):
    out, _ = _run(dict(
        support_X=support_X, query_X=query_X, m=m, kappa=kappa, nu=nu,
        triu_diag=triu_diag, triu_lower=triu_lower, labels=labels, way=way))
    return out


# revision 11
# speedup vs baseline: 1.5518x; 1.0421x over previous
"""MetaQDA Trainium2 kernel.

Strategy: Woodbury + matrix-determinant-lemma reformulation done on host
(fp64, tiny: one shared 768x768 inverse + 64 18x18 systems), reducing the
device work to dense matmuls per query shard:

  per query column q (768-vec):
    SQ   = M1 @ q                  (M1 [1920,768] = class rank-update rows + chol(P)^T)
    F[c] = Sel^T SQ^2 + Sel2^T q^2 + Coef^T q     (PSUM accumulation, [64] per q)
    out[c] = biasc[c] + lnmul[c] * ln(F[c]*invc[c] + blc[c])

Queries are sharded 8-way across NeuronCores (256 each); all class data is
replicated (it is only ~6.5MB of fp32 matrices). No collectives.
"""
import sys
import numpy as np
from math import lgamma

sys.path.insert(0, "/opt/trn_rl_repo")

REG = 0.5
D = 768
C = 64
NQ = 2048
NCORES = 8
QC = NQ // NCORES          # 256 queries per core
KPAD = 1152                # sum(1+Nc) = C+N = 1088, padded to 9*128
M1ROWS = KPAD + D          # 1920
KT_M1 = D // 128           # 6 contraction tiles for M1 @ q
MT_M1 = M1ROWS // 128      # 15 output tiles of SQ
SELTILES = M1ROWS // 128   # 15
F32 = np.float32


# ---------------------------------------------------------------- host math
def _precompute(support_X, query_X, m, kappa, nu, triu_diag, triu_lower,
                labels, way):
    d = D
    f64 = np.float64
    Xs = np.asarray(support_X, f64)
    mv = np.asarray(m, f64).reshape(-1)
    kap = abs(float(kappa)) + 1e-6
    nu_ = max(float(nu), d - 1 + 1e-6)
    way = int(way)

    Ld = np.abs(np.asarray(triu_diag, f64))
    L = np.diag(Ld) + np.tril(np.asarray(triu_lower, f64), -1)
    logdet_prior = 2.0 * np.sum(np.log(Ld))

    # P = inv(L L^T) via triangular solve in fp64
    Linv = _solve_tri_lower(L, np.eye(d))
    P = Linv.T @ Linv

    labels = np.asarray(labels).astype(np.int64)
    counts = np.bincount(labels, minlength=way).astype(f64)
    sums = np.zeros((way, d), f64)
    np.add.at(sums, labels, Xs)
    starts = np.searchsorted(labels, np.arange(way), side="left")
    ends = np.searchsorted(labels, np.arange(way), side="right")

    v = kap * mv[None, :] + sums
    mu = v / (kap + counts)[:, None]
    scale = (kap + counts + 1.0) / ((nu_ + counts - d + 1.0) * (kap + counts))
    alpha = (1.0 - REG) / scale
    common = nu_ + counts + 1.0 - d

    Pmu = mu @ P
    KXa = np.zeros((KPAD, d), f64)
    SelT = np.zeros((M1ROWS, way), f64)
    Sel2 = np.full((D, way), REG, f64)
    CoefT = np.zeros((d, way), f64)
    delta = np.zeros(way, f64)
    logdet = np.zeros(way, f64)

    row0 = 0
    for c in range(way):
        Nc = int(counts[c])
        Xc = Xs[starts[c]:ends[c]]
        nb = Nc + 2
        W = np.concatenate([mv[:, None], Xc.T, v[c][:, None]], axis=1)
        PW = P @ W
        Jdiag = np.concatenate([[kap], np.ones(Nc), [-1.0 / (kap + Nc)]])
        Mc = np.diag(1.0 / Jdiag) + W.T @ PW
        E = np.linalg.inv(Mc)
        _, ldM = np.linalg.slogdet(Mc)
        logdet[c] = (d * np.log(scale[c]) + logdet_prior + ldM
                     + np.sum(np.log(np.abs(Jdiag))))
        h = PW.T @ mu[c]
        Eh = E @ h
        T = np.zeros((nb, 1 + Nc))
        T[0, 0] = 1.0
        T[1:1 + Nc, 1:1 + Nc] = np.eye(Nc)
        T[nb - 1, 0] = kap
        T[nb - 1, 1:1 + Nc] = 1.0
        Fq = T.T @ E @ T
        g = T.T @ Eh
        kc = h @ Eh
        lam, V = np.linalg.eigh(Fq)
        R = (np.sqrt(np.abs(lam))[:, None]) * V.T
        Xhat = np.concatenate([mv[None, :], Xc], axis=0)
        KXa[row0:row0 + 1 + Nc] = (R @ Xhat) @ P
        SelT[row0:row0 + 1 + Nc, c] = -alpha[c] * np.sign(lam)
        row0 += 1 + Nc
        ghat = Xhat.T @ g
        CoefT[:, c] = (-2.0 * REG * mu[c] - 2.0 * alpha[c] * Pmu[c]
                       + 2.0 * alpha[c] * (P @ ghat))
        delta[c] = alpha[c] * (mu[c] @ Pmu[c] - kc) + REG * (mu[c] @ mu[c])

    abar = float(alpha.min())
    uniform = bool(np.allclose(alpha, abar, rtol=1e-12, atol=0.0))
    if uniform:
        # fold REG*||q||^2 into the shared quadratic block:
        # q^T (REG I + abar P) q  ->  rows chol(.)^T, Sel coefficient 1
        Cu = np.linalg.cholesky(REG * np.eye(d) + abar * P)
        SelT[KPAD:KPAD + D, :] = 1.0
        M1 = np.concatenate([KXa, Cu.T], axis=0)
    else:
        Lp = np.linalg.cholesky(P)
        SelT[KPAD:KPAD + D, :] = alpha[None, :]
        M1 = np.concatenate([KXa, Lp.T], axis=0)

    biasc = np.array([lgamma(0.5 * (common[c] + d)) - lgamma(0.5 * common[c])
                      - 0.5 * d * np.log(common[c]) - 0.5 * logdet[c]
                      for c in range(way)])
    invc = 1.0 / common
    lnmul = -0.5 * (common + d)
    blc = 1.0 + delta * invc

    pre = dict(
        M1T=np.ascontiguousarray(M1.T, dtype=F32),
        SelT=np.ascontiguousarray(SelT, dtype=F32),
        CoefT=np.ascontiguousarray(CoefT, dtype=F32),
        PC=np.ascontiguousarray(
            np.stack([invc, lnmul, biasc, blc], axis=1), dtype=F32),
    )
    if not uniform:
        pre["Sel2"] = np.ascontiguousarray(Sel2, dtype=F32)
    return pre


def _solve_tri_lower(L, B):
    try:
        import scipy.linalg as sla
        return sla.solve_triangular(L, B, lower=True)
    except ImportError:
        return np.linalg.solve(L, B)


# ---------------------------------------------------------------- device
_CACHE = {}


def _build_program(uniform):
    import concourse.bass as bass
    import concourse.bacc as bacc
    import concourse.tile as tile
    from concourse import mybir

    nc = bacc.Bacc("TRN2", target_bir_lowering=False, debug=False,
                   num_devices=NCORES)
    dt = mybir.dt.float32
    dtr = mybir.dt.float32r
    qT = nc.dram_tensor("qT", [D, QC], dt, kind="ExternalInput").ap()
    M1T = nc.dram_tensor("M1T", [D, M1ROWS], dt, kind="ExternalInput").ap()
    SelT = nc.dram_tensor("SelT", [M1ROWS, C], dt, kind="ExternalInput").ap()
    Sel2 = (None if uniform else
            nc.dram_tensor("Sel2", [D, C], dt, kind="ExternalInput").ap())
    CoefT = nc.dram_tensor("CoefT", [D, C], dt, kind="ExternalInput").ap()
    PC = nc.dram_tensor("PC", [C, 4], dt, kind="ExternalInput").ap()
    out = nc.dram_tensor("out", [C, QC], dt, kind="ExternalOutput").ap()

    with tile.TileContext(nc) as tc:
        with (
            tc.tile_pool(name="qpool", bufs=1) as qpool,
            tc.tile_pool(name="wpool", bufs=1) as wpool,
            tc.tile_pool(name="sqpool", bufs=1) as sqpool,
            tc.tile_pool(name="psum", bufs=6, space="PSUM") as psum,
            tc.tile_pool(name="fpsum", bufs=1, space="PSUM") as fpsum,
            tc.tile_pool(name="opool", bufs=1) as opool,
        ):
            dmaengs = [nc.sync, nc.gpsimd, nc.scalar, nc.sync]
            # load q^T tiles [128, QC] x 6 (small, gate first matmuls)
            qts = []
            for k in range(KT_M1):
                t = qpool.tile([128, QC], dtr, tag=f"qt{k}")
                dmaengs[k % 2].dma_start(
                    t[:], qT[k * 128:(k + 1) * 128, :].bitcast(dtr))
                qts.append(t)
            # big weight tiles spread round-robin over queue engines
            m1ts = []
            for k in range(KT_M1):
                t = wpool.tile([128, M1ROWS], dtr, tag=f"m1t{k}")
                dmaengs[k % 4].dma_start(
                    t[:], M1T[k * 128:(k + 1) * 128, :].bitcast(dtr))
                m1ts.append(t)
            selts = []
            for j in range(SELTILES):
                t = wpool.tile([128, C], dtr, tag=f"sel{j}")
                dmaengs[(j + 2) % 4].dma_start(
                    t[:], SelT[j * 128:(j + 1) * 128, :].bitcast(dtr))
                selts.append(t)
            sel2ts = []
            coefts = []
            for k in range(KT_M1):
                if not uniform:
                    t = wpool.tile([128, C], dtr, tag=f"sel2_{k}")
                    dmaengs[(k + 1) % 4].dma_start(
                        t[:], Sel2[k * 128:(k + 1) * 128, :].bitcast(dtr))
                    sel2ts.append(t)
                t2 = wpool.tile([128, C], dtr, tag=f"coef{k}")
                dmaengs[(k + 3) % 4].dma_start(
                    t2[:], CoefT[k * 128:(k + 1) * 128, :].bitcast(dtr))
                coefts.append(t2)
            pct = wpool.tile([C, 4], dt, tag="pc")
            nc.gpsimd.dma_start(pct[:], PC[:])

            # F psum [64, QC] accumulated across all matmuls
            fac = fpsum.tile([C, QC], dt, tag="facc")
            first = [True]

            def acc(lhsT, rhs, stop=False):
                nc.tensor.matmul(fac[:], lhsT, rhs, start=first[0], stop=stop)
                first[0] = False

            # SQ tiles: for each of 15 output tiles, 6-step contraction
            sqs = []
            for mtile in range(MT_M1):
                pt = psum.tile([128, QC], dt, tag="sqp")
                for k in range(KT_M1):
                    nc.tensor.matmul(
                        pt[:],
                        m1ts[k][:, mtile * 128:(mtile + 1) * 128],
                        qts[k][:],
                        start=(k == 0), stop=(k == KT_M1 - 1),
                    )
                sq = sqpool.tile([128, QC], dtr, tag=f"sqsq{mtile}")
                if mtile % 3 == 0:
                    nc.scalar.square(sq[:], pt[:])
                else:
                    cp = sqpool.tile([128, QC], dt, tag=f"sqcp{mtile}")
                    nc.vector.tensor_copy(cp[:], pt[:])
                    nc.vector.tensor_mul(sq[:], cp[:], cp[:])
                sqs.append(sq)

            # q^2 rows (DVE) — only needed when alpha varies per class
            q2s = []
            if not uniform:
                for k in range(KT_M1):
                    q2 = sqpool.tile([128, QC], dtr, tag=f"q2_{k}")
                    nc.vector.tensor_mul(q2[:], qts[k][:], qts[k][:])
                    q2s.append(q2)

            # accumulate everything into F
            for mtile in range(MT_M1):
                acc(selts[mtile][:], sqs[mtile][:])
            if not uniform:
                for k in range(KT_M1):
                    acc(sel2ts[k][:], q2s[k][:])
            for k in range(KT_M1):
                acc(coefts[k][:], qts[k][:], stop=(k == KT_M1 - 1))

            # epilogue: out = biasc + lnmul * ln(F*invc + blc)
            lnt = opool.tile([C, QC], dt, tag="lnt")
            nc.scalar.activation(lnt[:], fac[:],
                                 mybir.ActivationFunctionType.Ln,
                                 bias=pct[:, 3:4], scale=pct[:, 0:1])
            ot = opool.tile([C, QC], dt, tag="ot")
            nc.vector.tensor_scalar(ot[:], lnt[:], pct[:, 1:2], pct[:, 2:3],
                                    op0=mybir.AluOpType.mult,
                                    op1=mybir.AluOpType.add)
            nc.sync.dma_start(out[:], ot[:])

    nc.compile()
    return nc


def _run(inputs, **run_kwargs):
    from concourse.bass_utils import run_bass_kernel_spmd

    pre = _precompute(**inputs)
    uniform = "Sel2" not in pre
    key = ("nc", uniform)
    if key not in _CACHE:
        _CACHE[key] = _build_program(uniform)
    nc = _CACHE[key]

    qT_full = np.ascontiguousarray(np.asarray(inputs["query_X"], F32).T)
    in_maps = []
    for i in range(NCORES):
        im = dict(pre)
        im["qT"] = np.ascontiguousarray(qT_full[:, i * QC:(i + 1) * QC])
        in_maps.append(im)

    res = run_bass_kernel_spmd(nc, in_maps, core_ids=list(range(NCORES)),
                               **run_kwargs)
    outs = [res.results[i]["out"] for i in range(NCORES)]     # [64, 256] each
    full = np.concatenate(outs, axis=1)                        # [64, 2048]
    return np.ascontiguousarray(full.T.astype(F32)), res       # [2048, 64]


def kernel(support_X, query_X, m, kappa, nu, triu_diag, triu_lower,
           labels, way):
    out, _ = _run(dict(
        support_X=support_X, query_X=query_X, m=m, kappa=kappa, nu=nu,
        triu_diag=triu_diag, triu_lower=triu_lower, labels=labels, way=way))
    return out
